# revision 46
# baseline (speedup 1.0000x reference)
import sys

sys.path.insert(0, "/opt/trn_rl_repo")

import numpy as np

import concourse.bass as bass  # noqa: F401
import concourse.mybir as mybir
import concourse.tile as tile
from concourse import bacc
from concourse.bass_utils import run_bass_kernel_spmd
from concourse.masks import make_identity

B, C, HH, WW = 8, 256, 96, 96
N = HH * WW            # 9216
K = 64                 # phi out channels
S = 256                # num_state
NCHUNK = N // 128      # 72
NSLAB = 8
SLAB = N // NSLAB      # 1152
NBLK = N // 512        # 18
EPS_BN = 1e-5
EPS_NORM = 1e-12

f32 = mybir.dt.float32
f32r = mybir.dt.float32r
AF = mybir.ActivationFunctionType
AX = mybir.AxisListType
ALU = mybir.AluOpType

_cache = {}
_phase_ids = []


def _mark(nc, label):
    _phase_ids.append((label, nc.next_id()))


def _build(reps=1, loop_reps=1, stages='full'):
    nc = bacc.Bacc("TRN2", target_bir_lowering=False, debug=False, num_devices=8)
    X_d = nc.dram_tensor("X", [C, N], f32, kind="ExternalInput")
    PTW_d = nc.dram_tensor("PTW", [C, K + S], f32, kind="ExternalInput")
    ROW_d = nc.dram_tensor("ROW", [C, S], f32, kind="ExternalInput")
    VAW_d = nc.dram_tensor("VAW", [C, S], f32, kind="ExternalInput")
    Q_d = nc.dram_tensor("Q", [N, K // 2], f32, kind="ExternalOutput")
    Z_d = nc.dram_tensor("Z", [S, K], f32, kind="ExternalOutput")

    with tile.TileContext(nc) as tc:
        with (
            tc.tile_pool(name="res", bufs=1) as res,
            tc.tile_pool(name="stg", bufs=8) as stg,
            tc.tile_pool(name="wk", bufs=2) as wk,
            tc.tile_pool(name="ps", bufs=2, space="PSUM") as ps,
            tc.tile_pool(name="pacc", bufs=1, space="PSUM") as pacc,
        ):
            if loop_reps > 1:
                with tc.For_i(0, loop_reps, 1):
                    _emit_one(nc, res, stg, wk, ps, pacc,
                              X_d, PTW_d, ROW_d, VAW_d, Q_d, Z_d, stages)
            else:
                for _rep in range(reps):
                    _emit_one(nc, res, stg, wk, ps, pacc,
                              X_d, PTW_d, ROW_d, VAW_d, Q_d, Z_d, stages)

    nc.compile()
    return nc


def _emit_one(nc, res, stg, wk, ps, pacc, X_d, PTW_d, ROW_d, VAW_d, Q_d, Z_d, stages='full'):
    _mark(nc, 'setup')
    bf16 = mybir.dt.bfloat16
    # ---- constants ----
    ident = res.tile([128, 128], f32, tag="ident")
    make_identity(nc, ident[:])
    ones_f = res.tile([128, 2], f32, tag="ones_f")
    nc.gpsimd.memset(ones_f[:], 1.0)
    ones_f4 = res.tile([128, 4], f32, tag="ones_f4")
    nc.gpsimd.memset(ones_f4[:], 1.0)
    onesr_b = res.tile([128, 2], bf16, tag="ones_b")
    nc.vector.tensor_copy(onesr_b[:], ones_f[:])
    ones_row = res.tile([1, 128], f32, tag="ones_row")
    nc.gpsimd.memset(ones_row[:], 1.0)

    # ---- weights: DMA f32 staging -> convert to bf16 ----
    ptw_r, row_r, vaw_r = [], [], []
    for c in range(2):
        t = stg.tile([128, K + S], f32, tag="stg", name=f"wstg{c}")
        nc.sync.dma_start(t[:], PTW_d[c * 128:(c + 1) * 128, :])
        w = res.tile([128, K + S], bf16, tag=f"ptw{c}", name=f"ptw{c}")
        nc.vector.tensor_copy(w[:], t[:])
        ptw_r.append(w)
    for c in range(2):
        t = stg.tile([128, S], f32, tag="stg", name=f"rstg{c}")
        nc.sync.dma_start(t[:], ROW_d[c * 128:(c + 1) * 128, :])
        w = res.tile([128, S], bf16, tag=f"row{c}", name=f"row{c}")
        nc.gpsimd.tensor_copy(w[:], t[:])
        row_r.append(w)
    for c in range(2):
        t = stg.tile([128, S], f32, tag="stg", name=f"vstg{c}")
        nc.sync.dma_start(t[:], VAW_d[c * 128:(c + 1) * 128, :])
        w = res.tile([128, S], bf16, tag=f"vaw{c}", name=f"vaw{c}")
        nc.vector.tensor_copy(w[:], t[:])
        vaw_r.append(w)

    # ---- X load + bf16 conversion (resident) ----
    xr = [
        res.tile([128, N], bf16, tag=f"xr{c}", name=f"xr{c}")
        for c in range(2)
    ]
    for j in range(NSLAB):
        for c in range(2):
            t = stg.tile([128, SLAB], f32, tag="stg", name=f"xstg{j}_{c}")
            nc.sync.dma_start(
                t[:], X_d[c * 128:(c + 1) * 128, j * SLAB:(j + 1) * SLAB]
            )
            dst = xr[c][:, j * SLAB:(j + 1) * SLAB]
            if c == 0:
                nc.gpsimd.tensor_copy(dst, t[:])
            elif j % 2 == 0:
                nc.vector.tensor_copy(dst, t[:])
            else:
                nc.scalar.copy(dst, t[:])

    _mark(nc, 'load')
    if stages == 'load':
        zanchor = wk.tile([128, K], f32, tag="zanchor")
        nc.vector.tensor_copy(zanchor[:], xr[0][:, 0:K])
        nc.sync.dma_start(Z_d[0:128, :], zanchor[:])
        return
    # ---- phase 1: pt proj (2-chunk batches) -> e_theta/phi -> D0 ----
    # D0[k, s] = sum_n phi_aug[n, k] * e_theta[n, s];  row 64 = rowsum_theta
    d0_ps = pacc.tile([K + 1, S], f32, tag="acc", name="d0_ps")
    erou_all = [
        res.tile([128, N], bf16, tag=f"era{sc}", name=f"era{sc}")
        for sc in range(2)
    ]
    NG1 = NCHUNK // 2   # 36 groups of 2 chunks
    PIPE1 = 3           # groups of delay for d0
    p1ring = {}
    for g in range(NG1 + PIPE1):
        if g < NG1:
            t0 = 2 * g
            # pt2: two chunks in one 2-bank psum tile; chunk u at col u*512
            pt2 = ps.tile([128, 2, 512], f32, tag="big2", bufs=2, name="pt2")
            for u in range(2):
                t = t0 + u
                nc.tensor.matmul(pt2[:, u, 0:K + S],
                                 xr[0][:, t * 128:(t + 1) * 128],
                                 ptw_r[0][:], start=True, stop=False)
                nc.tensor.matmul(pt2[:, u, 0:K + S],
                                 xr[1][:, t * 128:(t + 1) * 128],
                                 ptw_r[1][:], start=False, stop=True)
            phia2 = wk.tile([128, 2, K + 1], bf16, tag="phia", bufs=5)
            nc.vector.tensor_scalar_max(phia2[:, :, 0:K], pt2[:, :, 0:K], 0.0)
            nc.vector.tensor_copy(phia2[:, :, K:K + 1],
                                  ones_f[:].rearrange("p (u k) -> p u k", k=1))
            thex2 = wk.tile([128, 2, S], bf16, tag="thex", bufs=5)
            nc.scalar.activation(thex2[:], pt2[:, :, K:K + S], AF.Exp)
            ethe2 = wk.tile([128, 2, S], bf16, tag="ethe", bufs=5)
            nc.vector.tensor_scalar_max(ethe2[:], thex2[:], 1.0)
            p1ring[g] = (phia2, ethe2)
        gd = g - PIPE1
        if gd >= 0:
            phia_d, ethe_d = p1ring.pop(gd)
            for u in range(2):
                td = 2 * gd + u
                nc.tensor.matmul(d0_ps[:], phia_d[:, u, :], ethe_d[:, u, :],
                                 start=(td == 0), stop=(td == NCHUNK - 1))

    _mark(nc, 'phase1')
    if stages == 'p1':
        zanchor = wk.tile([K + 1, K], f32, tag="zanchor")
        nc.vector.tensor_copy(zanchor[:], d0_ps[:, 0:K])
        nc.sync.dma_start(Z_d[0:K + 1, :], zanchor[:])
        return
    # ---- boundary 1: transpose D0, scale rows by 1/rowsum_theta ----
    # d0aug[sc] (128, 66) bf16: cols 0..63 = discrib.T, col 64 = 1, col 65 = 0
    d0sb = res.tile([K + 1, S], f32, tag="d0sb")
    nc.scalar.copy(d0sb[:], d0_ps[:])
    d0aug = []
    for sc in range(2):
        tp_ps = ps.tile([128, K + 1], f32, tag="qt", bufs=3, name=f"tp{sc}")
        nc.tensor.transpose(tp_ps[:], d0sb[:, sc * 128:(sc + 1) * 128],
                            ident[0:K + 1, 0:K + 1])
        rth = wk.tile([128, 1], f32, tag="rth")
        nc.vector.reciprocal(rth[:], tp_ps[:, K:K + 1])
        stage = wk.tile([128, K + 2], f32, tag="dstage")
        nc.gpsimd.memset(stage[:], 0.0)
        nc.gpsimd.memset(stage[:, K:K + 1], 1.0)
        nc.scalar.activation(stage[:, 0:K], tp_ps[:, 0:K], AF.Copy,
                             bias=0.0, scale=rth[:])
        da = res.tile([128, K + 2], bf16, tag=f"d0aug{sc}", name=f"d0aug{sc}")
        nc.vector.tensor_copy(da[:], stage[:])
        d0aug.append(da)

    _mark(nc, 'bound1')
    # ---- phase 2a: rou proj -> e_rou; QT (4-chunk batches) -> Q0, sumsq ----
    q0all = res.tile([128, NCHUNK * K], bf16, tag="q0all")
    ssq_ps = pacc.tile([1, 4 * K], f32, tag="acc", name="ssq_ps")

    def emit_rou_block(j):
        for sc in range(2):
            rou_ps = ps.tile([128, 2, 512], f32, tag="big2", bufs=2,
                             name=f"rou{sc}")
            nc.tensor.matmul(rou_ps[:, 0, :],
                             row_r[0][:, sc * 128:(sc + 1) * 128],
                             xr[0][:, j * 512:(j + 1) * 512],
                             start=True, stop=False)
            nc.tensor.matmul(rou_ps[:, 0, :],
                             row_r[1][:, sc * 128:(sc + 1) * 128],
                             xr[1][:, j * 512:(j + 1) * 512],
                             start=False, stop=True)
            roex = wk.tile([128, 512], bf16, tag="roex", bufs=4,
                           name=f"roex{sc}")
            nc.scalar.activation(roex[:], rou_ps[:, 0, :], AF.Exp)
            nc.vector.tensor_scalar_max(
                erou_all[sc][:, j * 512:(j + 1) * 512], roex[:], 1.0)

    ssqring = {}
    emit_rou_block(0)
    for j in range(NBLK):
        if j + 1 < NBLK:
            emit_rou_block(j + 1)
        qt4 = ps.tile([128, 4, K + 2], f32, tag="qt", bufs=3, name="qt4")
        for u in range(4):
            t = j * 4 + u
            nc.tensor.matmul(qt4[:, u, :],
                             erou_all[0][:, t * 128:(t + 1) * 128],
                             d0aug[0][:], start=True, stop=False)
            nc.tensor.matmul(qt4[:, u, :],
                             erou_all[1][:, t * 128:(t + 1) * 128],
                             d0aug[1][:], start=False, stop=True)
        rc4 = wk.tile([128, 4], f32, tag="rc", bufs=3)
        nc.vector.reciprocal(rc4[:], qt4[:, :, K])
        for u in range(4):
            t = j * 4 + u
            nc.vector.tensor_scalar_mul(q0all[:, t * K:(t + 1) * K],
                                        qt4[:, u, 0:K], rc4[:, u:u + 1])
        sqr4 = wk.tile([128, 4 * K], bf16, tag="sqr", bufs=3)
        nc.vector.tensor_mul(sqr4[:], q0all[:, j * 4 * K:(j + 1) * 4 * K],
                             q0all[:, j * 4 * K:(j + 1) * 4 * K])
        ssqring[j] = sqr4
        jd = j - 2
        if jd >= 0:
            sq_d = ssqring.pop(jd)
            nc.tensor.matmul(ssq_ps[:], onesr_b[:, 0:1], sq_d[:],
                             start=(jd == 0), stop=(jd == NBLK - 1))
    for jd in sorted(ssqring):
        sq_d = ssqring.pop(jd)
        nc.tensor.matmul(ssq_ps[:], onesr_b[:, 0:1], sq_d[:],
                         start=(jd == 0), stop=(jd == NBLK - 1))

    _mark(nc, 'phase2a')
    if stages == 'p2a':
        zanchor = wk.tile([128, K], f32, tag="zanchor")
        nc.vector.tensor_copy(zanchor[:], q0all[:, 0:K])
        nc.sync.dma_start(Z_d[0:128, :], zanchor[:])
        return
    # ---- boundary 2: rscale = 1/max(sqrt(sum ssq), eps), bcast ----
    ssqs = wk.tile([1, 4 * K], f32, tag="ssqs")
    nc.vector.tensor_copy(ssqs[:], ssq_ps[:])
    rsum = wk.tile([1, K], f32, tag="rs0")
    nc.vector.tensor_add(rsum[:], ssqs[:, 0:K], ssqs[:, K:2 * K])
    nc.vector.tensor_add(rsum[:], rsum[:], ssqs[:, 2 * K:3 * K])
    nc.vector.tensor_add(rsum[:], rsum[:], ssqs[:, 3 * K:4 * K])
    rs1 = wk.tile([1, K], f32, tag="rs1")
    nc.scalar.activation(rs1[:], rsum[:], AF.Sqrt)
    rs2 = wk.tile([1, K], f32, tag="rs2")
    nc.vector.tensor_scalar_max(rs2[:], rs1[:], EPS_NORM)
    rs34 = wk.tile([1, 4 * K], f32, tag="rs34")
    for u in range(4):
        nc.vector.reciprocal(rs34[:, u * K:(u + 1) * K], rs2[:])
    bc_ps = ps.tile([128, 4 * K], f32, tag="qt", bufs=3, name="bc_ps")
    nc.tensor.matmul(bc_ps[:], ones_row[:], rs34[:], start=True, stop=True)
    rsbc4 = res.tile([128, 4 * K], bf16, tag="rsbc4")
    nc.scalar.copy(rsbc4[:], bc_ps[:])

    _mark(nc, 'bound2')
    # ---- phase 2b: softmax_k(Q0 * rscale) -> Q out; val; Znum+qsum ----
    # No max-subtraction needed: |Q1| <= 1 after the L2 normalization.
    # qsum folded into zn via two ones-columns appended to valr.
    zn_ps = pacc.tile([K, S + 2], f32, tag="acc", name="zn_ps")

    def emit_2b_head(j):
        q14 = wk.tile([128, 4 * K], bf16, tag="q14", bufs=3)
        nc.vector.tensor_mul(q14[:], q0all[:, j * 4 * K:(j + 1) * 4 * K],
                             rsbc4[:])
        ee4 = wk.tile([128, 4 * K], bf16, tag="ee4", bufs=3)
        nc.scalar.activation(ee4[:], q14[:], AF.Exp)
        den4 = wk.tile([128, 4], f32, tag="den4", bufs=3)
        nc.vector.tensor_reduce(
            den4[:], ee4[:].rearrange("p (u k) -> p u k", k=K),
            axis=AX.X, op=ALU.add)
        rden4 = wk.tile([128, 4], f32, tag="rden4", bufs=3)
        nc.vector.reciprocal(rden4[:], den4[:])
        return ee4, rden4

    PIPE3 = 2
    znring = {}
    head = emit_2b_head(0)
    for j in range(NBLK):
        ee4, rden4 = head
        if j + 1 < NBLK:
            head = emit_2b_head(j + 1)
        q2b4 = wk.tile([128, 4, K], bf16, tag="q2b4", bufs=3)
        for h in range(2):
            t0 = j * 4 + 2 * h
            val2 = ps.tile([128, 2, S], f32, tag="qt", bufs=3, name="val2")
            for u2 in range(2):
                t = t0 + u2
                nc.tensor.matmul(val2[:, u2, :],
                                 xr[0][:, t * 128:(t + 1) * 128],
                                 vaw_r[0][:], start=True, stop=False)
                nc.tensor.matmul(val2[:, u2, :],
                                 xr[1][:, t * 128:(t + 1) * 128],
                                 vaw_r[1][:], start=False, stop=True)
            valr2 = wk.tile([128, 2, S + 2], bf16, tag="valr", bufs=4)
            if h == 0:
                nc.scalar.activation(valr2[:, :, 0:S], val2[:], AF.Relu)
            else:
                nc.vector.tensor_scalar_max(valr2[:, :, 0:S], val2[:], 0.0)
            nc.vector.tensor_copy(
                valr2[:, :, S:S + 2],
                ones_f4[:].rearrange("p (u k) -> p u k", k=2))
            for u2 in range(2):
                u = 2 * h + u2
                t = t0 + u2
                nc.vector.tensor_scalar_mul(q2b4[:, u, :],
                                            ee4[:, u * K:(u + 1) * K],
                                            rden4[:, u:u + 1])
                znring[t] = (q2b4, u, valr2, u2)
                td = t - PIPE3
                if td >= 0:
                    q2b_d, uq, valr_d, ud = znring.pop(td)
                    nc.tensor.matmul(zn_ps[:], q2b_d[:, uq, :],
                                     valr_d[:, ud, :],
                                     start=(td == 0),
                                     stop=(td == NCHUNK - 1))
        if stages != 'nodma':
            nc.sync.dma_start(
                Q_d[j * 512:(j + 1) * 512, :].rearrange("(u p) k -> p u k",
                                                        p=128),
                q2b4[:].bitcast(f32))
    for td in sorted(znring):
        q2b_d, uq, valr_d, ud = znring.pop(td)
        nc.tensor.matmul(zn_ps[:], q2b_d[:, uq, :], valr_d[:, ud, :],
                         start=(td == 0), stop=(td == NCHUNK - 1))

    _mark(nc, 'phase2b')
    # ---- phase 3: Z = l2norm_s(Znum / qsum), transpose, out ----
    rq = wk.tile([K, 1], f32, tag="rq")
    nc.vector.reciprocal(rq[:], zn_ps[:, S:S + 1])
    z0 = wk.tile([K, S], f32, tag="z0")
    nc.scalar.activation(z0[:], zn_ps[:, 0:S], AF.Copy, bias=0.0, scale=rq[:])
    zsq = wk.tile([K, S], f32, tag="zsq")
    nc.vector.tensor_mul(zsq[:], z0[:], z0[:])
    zs = wk.tile([K, 1], f32, tag="zs")
    nc.vector.tensor_reduce(zs[:], zsq[:], axis=AX.X, op=ALU.add)
    zn1 = wk.tile([K, 1], f32, tag="zn1")
    nc.scalar.activation(zn1[:], zs[:], AF.Sqrt)
    zn2 = wk.tile([K, 1], f32, tag="zn2")
    nc.vector.tensor_scalar_max(zn2[:], zn1[:], EPS_NORM)
    rz = wk.tile([K, 1], f32, tag="rz")
    nc.vector.reciprocal(rz[:], zn2[:])
    zf = wk.tile([K, S], f32, tag="zf")
    nc.scalar.activation(zf[:], z0[:], AF.Copy, bias=0.0, scale=rz[:])
    for sc in range(2):
        zt_ps = ps.tile([128, K], f32, tag="qt", bufs=3, name=f"ztp{sc}")
        nc.tensor.transpose(zt_ps[:], zf[:, sc * 128:(sc + 1) * 128],
                            ident[0:K, 0:K])
        zt = wk.tile([128, K], f32, tag="zt", name=f"zt{sc}")
        nc.scalar.copy(zt[:], zt_ps[:])
        nc.sync.dma_start(Z_d[sc * 128:(sc + 1) * 128, :], zt[:])
    _finish_marks(nc)


def _finish_marks(nc):
    _mark(nc, 'end')


def _get_nc():
    if "nc" not in _cache:
        _cache["nc"] = _build()
    return _cache["nc"]


def _fold(w, g, b, m, v):
    s = (g / np.sqrt(v + EPS_BN)).astype(np.float32)
    bias = (b - m * s).astype(np.float32)
    if not np.allclose(bias, 0.0, atol=1e-30):
        raise ValueError("nonzero effective BN bias not supported")
    return (w.astype(np.float32) * s[:, None]).astype(np.float32)


def _run(inputs, trace=False, trace_kwargs=None):
    a = {k: np.asarray(v, dtype=np.float32) for k, v in inputs.items()}
    X = np.ascontiguousarray(a["X"]).reshape(B, C, N)
    phiW = _fold(a["phi_w"], a["phi_g"], a["phi_b"], a["phi_m"], a["phi_v"])
    thW = _fold(a["theta_w"], a["theta_g"], a["theta_b"], a["theta_m"], a["theta_v"])
    roW = _fold(a["rou_w"], a["rou_g"], a["rou_b"], a["rou_m"], a["rou_v"])
    vaW = _fold(a["val_w"], a["val_g"], a["val_b"], a["val_m"], a["val_v"])
    PTW = np.ascontiguousarray(np.concatenate([phiW.T, thW.T], axis=1))
    ROW = np.ascontiguousarray(roW.T)
    VAW = np.ascontiguousarray(vaW.T)

    nc = _get_nc()
    in_maps = [
        {"X": np.ascontiguousarray(X[i]), "PTW": PTW, "ROW": ROW, "VAW": VAW}
        for i in range(B)
    ]
    res = run_bass_kernel_spmd(
        nc, in_maps, core_ids=list(range(8)), trace=trace,
        **(dict(trace_kwargs=trace_kwargs) if trace_kwargs else {}),
    )
    Z = np.stack([res.results[i]["Z"] for i in range(B)]).astype(np.float32)
    import ml_dtypes
    Q = np.stack([
        np.ascontiguousarray(np.asarray(res.results[i]["Q"]))
        .view(ml_dtypes.bfloat16).astype(np.float32)
        for i in range(B)
    ])
    return (Z, Q), res


def kernel(**inputs):
    out, _ = _run(inputs, trace=False)
    return out
